# revision 8
# baseline (speedup 1.0000x reference)
"""Trainium2 Bass kernel for AttentionLateralOp.

Reference computation (per batch b):
    x = origin_out[b].reshape(C, N)      # keys/values source
    t = target_in[b].reshape(C, N)       # queries source + residual
    f = Wq @ t          [CQK, N]
    g = Wk @ x          [CQK, N]
    v = Wv @ x          [C, N]
    scores = f^T @ g    [N, N]
    beta = softmax(scores, axis=0)       # over i (rows)
    o = gamma * v @ beta + t

Sharding: 8 cores = (batch b = core//2) x (half of the j/output axis =
core%2). Each core computes the full f and v^T for its batch, and the
j-shard of g / scores / output.

Softmax-over-the-contraction-axis trick: append a ones row to f and a
(-mhat_j) row to g, so the PE emits max-subtracted logits directly into
PSUM; Z_j comes from a ones-vector matmul over E; the final gamma/Z_j
scaling and +t residual are per-partition ops in the transposed [j, c]
output orientation (output is transposed back on the host).
"""

import sys

sys.path.insert(0, "/opt/trn_rl_repo")

import numpy as np

import concourse.bass as bass  # noqa: F401  (bass types via bacc)
import concourse.tile as tile
from concourse import bacc, mybir
from concourse.bass import ds, ts
from concourse.bass_utils import run_bass_kernel_spmd
from concourse.masks import make_identity

F32 = mybir.dt.float32
F32R = mybir.dt.float32r
AF = mybir.ActivationFunctionType
ALU = mybir.AluOpType
AX = mybir.AxisListType

B, C, H, W = 4, 512, 64, 64
N = H * W            # 4096
CQK = C // 8         # 64
NCORES = 8
NJ = B * N // NCORES  # 2048 columns of the j axis per core
JT = 256             # j-tile width in the main loop
NIC = N // 128       # 32 i-chunks
NCC = C // 128       # 4 contraction chunks over C


def _build():
    nc = bacc.Bacc(None, target_bir_lowering=False)

    x_d = nc.dram_tensor("x", [C, N], F32, kind="ExternalInput")
    t_d = nc.dram_tensor("t", [C, N], F32, kind="ExternalInput")
    xg_d = nc.dram_tensor("xg", [C, NJ], F32, kind="ExternalInput")
    ttr_d = nc.dram_tensor("ttr", [NJ, C], F32, kind="ExternalInput")
    wqt_d = nc.dram_tensor("wqt", [C, CQK], F32, kind="ExternalInput")
    wkt_d = nc.dram_tensor("wkt", [C, CQK], F32, kind="ExternalInput")
    wvt_d = nc.dram_tensor("wvt", [C, C], F32, kind="ExternalInput")
    gam_d = nc.dram_tensor("gam", [128, 1], F32, kind="ExternalInput")
    o_d = nc.dram_tensor("o", [NJ, C], F32, kind="ExternalOutput")

    with tile.TileContext(nc) as tc:
        with tc.tile_pool(name="persist", bufs=1) as persist:
            # v^T with an appended ones column (column C) for Z
            vt = persist.tile([128, NIC, C + 1], F32R)
            # f with an appended ones row (row CQK) for the -mhat shift
            fp = persist.tile([CQK + 1, N], F32R)
            # g with an appended -mhat row (row CQK)
            gp = persist.tile([CQK + 1, NJ], F32R)
            ident = persist.tile([128, 128], F32)
            ones_col = persist.tile([128, 1], F32R)
            mall = persist.tile([128, 16], F32)
            nmneg = persist.tile([16, 128], F32R)
            gam_sb = persist.tile([128, 1], F32)

            make_identity(nc, ident)
            nc.sync.dma_start(gam_sb, gam_d[:])
            # f32r tiles can't be memset directly; Copy(x*0 + 1) writes 1.0
            nc.scalar.activation(
                ones_col, gam_sb, AF.Copy, bias=1.0, scale=0.0
            )

            with (
                tc.tile_pool(name="wpool", bufs=1) as wpool,
                tc.tile_pool(name="xstream", bufs=10) as xstream,
                tc.tile_pool(name="tstream", bufs=6) as tstream,
                tc.tile_pool(name="psA", bufs=2, space="PSUM") as psA,
                tc.tile_pool(name="psV", bufs=2, space="PSUM") as psV,
            ):
                wqt_sb = wpool.tile([128, NCC, CQK], F32R)
                wkt_sb = wpool.tile([128, NCC, CQK], F32R)
                wvt_sb = wpool.tile([128, NCC, C], F32R)
                for cc in range(NCC):
                    nc.sync.dma_start(
                        wqt_sb[:, cc, :], wqt_d[ts(cc, 128), :].bitcast(F32R)
                    )
                    nc.sync.dma_start(
                        wkt_sb[:, cc, :], wkt_d[ts(cc, 128), :].bitcast(F32R)
                    )
                    nc.sync.dma_start(
                        wvt_sb[:, cc, :], wvt_d[ts(cc, 128), :].bitcast(F32R)
                    )

                # f = Wq @ t  -> [CQK, N]
                for it in range(N // 512):
                    pf = psA.tile([CQK, 512], F32, tag="ps_scratch")
                    for cc in range(NCC):
                        tt = tstream.tile([128, 512], F32R, name="tt")
                        nc.sync.dma_start(
                            tt, t_d[ts(cc, 128), ts(it, 512)].bitcast(F32R)
                        )
                        nc.tensor.matmul(
                            pf,
                            wqt_sb[:, cc, :],
                            tt,
                            start=(cc == 0),
                            stop=(cc == NCC - 1),
                        )
                    nc.scalar.copy(fp[0:CQK, ts(it, 512)], pf)
                    nc.scalar.activation(
                        fp[CQK : CQK + 1, ts(it, 512)],
                        pf[0:1, :],
                        AF.Copy,
                        bias=1.0,
                        scale=0.0,
                    )

                # g = Wk @ x[:, jshard]  -> [CQK, NJ]
                for jt4 in range(NJ // 512):
                    pg = psA.tile([CQK, 512], F32, tag="ps_scratch")
                    for cc in range(NCC):
                        xgt = tstream.tile([128, 512], F32R, name="xgt")
                        nc.sync.dma_start(
                            xgt, xg_d[ts(cc, 128), ts(jt4, 512)].bitcast(F32R)
                        )
                        nc.tensor.matmul(
                            pg,
                            wkt_sb[:, cc, :],
                            xgt,
                            start=(cc == 0),
                            stop=(cc == NCC - 1),
                        )
                    nc.scalar.copy(gp[0:CQK, ts(jt4, 512)], pg)

                # pass 1 (subsampled): scores^T [j, i_sub] -> row max mhat
                fsub = fp[0:CQK, :].rearrange("p (n e) -> p n e", e=8)[:, :, 0:1]
                for jc in range(NJ // 128):
                    ps1 = psA.tile([128, 512], F32, tag="ps_scratch")
                    nc.tensor.matmul(
                        ps1, gp[0:CQK, ts(jc, 128)], fsub, start=True, stop=True
                    )
                    nc.vector.reduce_max(mall[:, jc : jc + 1], ps1, axis=AX.X)

                # transpose mhat [128,16] -> [16,128], negate, write g row CQK
                pmt = psA.tile([16, 128], F32, tag="ps_scratch")
                nc.tensor.matmul(pmt, mall, ident, start=True, stop=True)
                nc.scalar.mul(nmneg, pmt, -1.0)
                for k in range(16):
                    nc.sync.dma_start(
                        gp[CQK : CQK + 1, ts(k, 128)], nmneg[k : k + 1, :]
                    )

                # v^T = (Wv @ x)^T -> [N, C] (+ones col), computed directly
                for ic in range(NIC):
                    pv = psV.tile([128, C], F32)
                    for cc in range(NCC):
                        xt = xstream.tile([128, 128], F32R, name="xt")
                        nc.sync.dma_start(
                            xt, x_d[ts(cc, 128), ts(ic, 128)].bitcast(F32R)
                        )
                        nc.tensor.matmul(
                            pv,
                            xt,
                            wvt_sb[:, cc, :],
                            start=(cc == 0),
                            stop=(cc == NCC - 1),
                        )
                    nc.vector.tensor_copy(vt[:, ic, 0:C], pv)
                    nc.scalar.activation(
                        vt[:, ic, C : C + 1],
                        pv[:, 0:1],
                        AF.Copy,
                        bias=1.0,
                        scale=0.0,
                    )

            # main loop over j-tiles
            with (
                tc.tile_pool(name="epool", bufs=2) as epool,
                tc.tile_pool(name="ttrp", bufs=3) as ttrp,
                tc.tile_pool(name="obp", bufs=3) as obp,
                tc.tile_pool(name="zp", bufs=2) as zp,
                tc.tile_pool(name="pssc", bufs=3, space="PSUM") as pssc,
                tc.tile_pool(name="pso", bufs=2, space="PSUM") as pso,
                tc.tile_pool(name="psz", bufs=1, space="PSUM") as psz,
                tc.tile_pool(name="zdram", bufs=2, space="DRAM") as zdram,
            ):
                for jt in range(NJ // JT):
                    E = epool.tile([128, NIC, JT], F32R, name="E")
                    pz = psz.tile([1, JT], F32, name="pz")
                    for ic in range(NIC):
                        psc = pssc.tile([128, JT], F32)
                        nc.tensor.matmul(
                            psc,
                            fp[:, ts(ic, 128)],
                            gp[:, ts(jt, JT)],
                            start=True,
                            stop=True,
                        )
                        nc.scalar.activation(E[:, ic, :], psc, AF.Exp)
                        nc.tensor.matmul(
                            pz,
                            ones_col,
                            E[:, ic, :],
                            start=(ic == 0),
                            stop=(ic == NIC - 1),
                        )
                    zrow = zp.tile([1, JT], F32, name="zrow")
                    nc.scalar.copy(zrow, pz)
                    # transpose the Z row into a [128, 2] column tile via DRAM
                    zd = zdram.tile([1, JT], F32, name="zd")
                    nc.sync.dma_start(zd[:], zrow[:])
                    zcol = zp.tile([128, JT // 128], F32, name="zcol")
                    nc.sync.dma_start(
                        zcol[:], zd[0, :].rearrange("(c p) -> p c", p=128)
                    )
                    for jc2 in range(JT // 128):
                        j0 = jt * JT + jc2 * 128
                        po = pso.tile([128, C], F32, name="po")
                        for ic in range(NIC):
                            nc.tensor.matmul(
                                po,
                                E[:, ic, ts(jc2, 128)],
                                vt[:, ic, 0:C],
                                start=(ic == 0),
                                stop=(ic == NIC - 1),
                            )
                        zinv = zp.tile([128, 1], F32, name="zinv")
                        nc.vector.reciprocal(zinv, zcol[:, jc2 : jc2 + 1])
                        nc.vector.tensor_mul(zinv, zinv, gam_sb)
                        ttt = ttrp.tile([128, C], F32, name="ttt")
                        nc.sync.dma_start(ttt, ttr_d[ds(j0, 128), :])
                        ob = obp.tile([128, C], F32, name="ob")
                        nc.vector.scalar_tensor_tensor(
                            ob, po, zinv, ttt, op0=ALU.mult, op1=ALU.add
                        )
                        nc.sync.dma_start(o_d[ds(j0, 128), :], ob)

    nc.compile()
    return nc


_NC_CACHE = None


def _get_nc():
    global _NC_CACHE
    if _NC_CACHE is None:
        _NC_CACHE = _build()
    return _NC_CACHE


def make_in_maps(origin_out, target_in, Wq, Wk, Wv, gamma):
    x_b = np.ascontiguousarray(
        np.asarray(origin_out, dtype=np.float32).reshape(B, C, N)
    )
    t_b = np.ascontiguousarray(
        np.asarray(target_in, dtype=np.float32).reshape(B, C, N)
    )
    wqt = np.ascontiguousarray(np.asarray(Wq, dtype=np.float32).T)
    wkt = np.ascontiguousarray(np.asarray(Wk, dtype=np.float32).T)
    wvt = np.ascontiguousarray(np.asarray(Wv, dtype=np.float32).T)
    gam = np.full((128, 1), np.asarray(gamma, dtype=np.float32).reshape(-1)[0],
                  dtype=np.float32)
    in_maps = []
    for core in range(NCORES):
        b, half = core // 2, core % 2
        j0 = half * NJ
        in_maps.append(
            {
                "x": x_b[b],
                "t": t_b[b],
                "xg": np.ascontiguousarray(x_b[b][:, j0 : j0 + NJ]),
                "ttr": np.ascontiguousarray(t_b[b][:, j0 : j0 + NJ].T),
                "wqt": wqt,
                "wkt": wkt,
                "wvt": wvt,
                "gam": gam,
            }
        )
    return in_maps


def run_cores(in_maps, **kwargs):
    nc = _get_nc()
    return run_bass_kernel_spmd(nc, in_maps, core_ids=list(range(NCORES)), **kwargs)


def assemble(results):
    o = np.empty((B, C, N), dtype=np.float32)
    for core in range(NCORES):
        b, half = core // 2, core % 2
        j0 = half * NJ
        o[b][:, j0 : j0 + NJ] = results[core]["o"].T
    return o.reshape(B, C, H, W)


def kernel(origin_out, target_in, Wq, Wk, Wv, gamma):
    in_maps = make_in_maps(origin_out, target_in, Wq, Wk, Wv, gamma)
    res = run_cores(in_maps)
    return assemble(res.results)
